# revision 2
# baseline (speedup 1.0000x reference)
"""Differentiable 3D Gaussian renderer on 8 Trainium2 NeuronCores.

Math (per batch b):
    R = quat_to_rot(qvec[b]);  p_cam = positions @ R.T + tvec[b]
    X = p_cam.x / p_cam.z * FX + CX ;  Y likewise
    w[n, p] = opacity_n * exp(-0.5 * ((px - X_n)^2 + (py - Y_n)^2) / scales_n^2)
    img[p] = (sum_n w * color_n) / (sum_n w + 1e-8)

Key restructuring for the hardware: the Gaussian is separable,
    w[n, (x, y)] = wx[n, x] * wy[n, y] * opacity_n
with wx/wy being [N, 128] tables (pixel coordinates take only 128 distinct
values per axis).  Per 128-gaussian tile the whole pixel-space reduction
becomes ONE 128x128x512 fp32 matmul:
    out[y, (x, c)] += sum_n wy[n, y] * (wx[n, x] * colors4op[n, c])
with colors4op = [r*op, g*op, b*op, op] (c=3 accumulates the denominator).

Sharding: 2 batches x 4-way gaussian shard (1024 gaussians/core).  Each group
of 4 cores ReduceScatters its partial [128, 512] accumulator (each rank gets
32 image rows), divides locally, and returns a [3, 32, 128] image slice.
The host stacks the 8 slices into the [2, 3, 128, 128] output.
"""
import sys

for _p in ("/opt/trn_rl_repo", "/root/.axon_site/_ro/trn_rl_repo"):
    if _p not in sys.path:
        sys.path.append(_p)

import numpy as np

import concourse.bass as bass
import concourse.bacc as bacc
import concourse.tile as tile
from concourse import mybir
from concourse.bass_utils import run_bass_kernel_spmd

F32 = mybir.dt.float32
ALU = mybir.AluOpType
ACTF = mybir.ActivationFunctionType

H = W = 128
FX = 500.0
CX = 64.0
EPS = 1e-8
N_FULL = 4096
NCORES = 8
NSHARD = N_FULL // 4          # 1024 gaussians per core
NGT = NSHARD // 128           # 8 gaussian tiles per core
GROUPS = [[0, 1, 2, 3], [4, 5, 6, 7]]

_NC_CACHE = None


def build_nc():
    nc = bacc.Bacc("TRN2", target_bir_lowering=False, debug=False,
                   num_devices=NCORES)

    gdat = nc.dram_tensor("gdat", [8, NSHARD], F32, kind="ExternalInput")
    cam = nc.dram_tensor("cam", [12], F32, kind="ExternalInput")
    img = nc.dram_tensor("img_part", [3, 32, 128], F32, kind="ExternalOutput")

    idx_np = np.tile(np.arange(128, dtype=np.float32), (128, 1))
    idx_const = nc.inline_tensor(idx_np, "idxrep")

    with tile.TileContext(nc) as tc:
        with (
            tc.tile_pool(name="singles", bufs=1) as singles,
            tc.tile_pool(name="pre", bufs=1) as pre,
            tc.tile_pool(name="work", bufs=3) as work,
            tc.tile_pool(name="ps", bufs=1, space="PSUM") as ps,
            tc.tile_pool(name="dram", bufs=1, space="DRAM") as dram,
        ):
            # ---- load inputs -------------------------------------------------
            # gsb[p, 8k + s] = gdat[k, s*128 + p]
            gsb = singles.tile([128, 64], F32)
            g0 = gdat.ap()
            nc.sync.dma_start(
                out=bass.AP(gsb[:].tensor, gsb[:].offset,
                            [gsb[:].ap[0], [8, 8], [1, 8]]),
                in_=bass.AP(g0.tensor, 0, [[1, 128], [NSHARD, 8], [128, 8]]),
            )
            cam_sb = singles.tile([128, 12], F32)
            c0 = cam.ap()
            nc.sync.dma_start(
                out=cam_sb[:],
                in_=bass.AP(c0.tensor, 0, [[0, 128], [1, 12]]),
            )
            idx_sb = singles.tile([128, 128], F32)
            nc.sync.dma_start(out=idx_sb[:], in_=idx_const.ap())

            def gcol(k):
                return gsb[:, 8 * k:8 * (k + 1)]

            posx, posy, posz = gcol(0), gcol(1), gcol(2)
            colr, colg, colb = gcol(3), gcol(4), gcol(5)
            opac, scal = gcol(6), gcol(7)

            def cs(i):
                return cam_sb[:, i:i + 1]

            # ---- projection preamble ([128, 8] tiles) ------------------------
            def project(name, px, py, pz, r0, r1, r2, tt):
                t1 = pre.tile([128, 8], F32, tag=f"{name}_t1")
                nc.vector.tensor_scalar(t1[:], px, cs(r0), cs(tt),
                                        ALU.mult, ALU.add)
                t2 = pre.tile([128, 8], F32, tag=f"{name}_t2")
                nc.vector.scalar_tensor_tensor(t2[:], py, cs(r1), t1[:],
                                               ALU.mult, ALU.add)
                t3 = pre.tile([128, 8], F32, tag=f"{name}_t3")
                nc.vector.scalar_tensor_tensor(t3[:], pz, cs(r2), t2[:],
                                               ALU.mult, ALU.add)
                return t3

            pcx = project("pcx", posx, posy, posz, 0, 1, 2, 9)
            pcy = project("pcy", posx, posy, posz, 3, 4, 5, 10)
            pcz = project("pcz", posx, posy, posz, 6, 7, 8, 11)

            rz = pre.tile([128, 8], F32)
            nc.vector.reciprocal(rz[:], pcz[:])

            tX = pre.tile([128, 8], F32)
            nc.vector.tensor_mul(tX[:], pcx[:], rz[:])
            negX = pre.tile([128, 8], F32)
            nc.vector.tensor_scalar(negX[:], tX[:], -FX, CX,
                                    ALU.mult, ALU.subtract)
            tY = pre.tile([128, 8], F32)
            nc.vector.tensor_mul(tY[:], pcy[:], rz[:])
            negY = pre.tile([128, 8], F32)
            nc.vector.tensor_scalar(negY[:], tY[:], -FX, CX,
                                    ALU.mult, ALU.subtract)

            rsc = pre.tile([128, 8], F32)
            nc.vector.reciprocal(rsc[:], scal)
            salpha = pre.tile([128, 8], F32)
            nc.vector.tensor_scalar_mul(salpha[:], rsc[:],
                                        float(np.sqrt(0.5)))

            # colors4op[p, 4s + c] = [r*op, g*op, b*op, op][c]
            col4o = pre.tile([128, 32], F32)
            c4 = col4o[:]
            for off, ch in ((0, colr), (1, colg), (2, colb)):
                nc.vector.tensor_mul(
                    bass.AP(c4.tensor, c4.offset + off, [c4.ap[0], [4, 8]]),
                    ch, opac)
            nc.vector.tensor_copy(
                bass.AP(c4.tensor, c4.offset + 3, [c4.ap[0], [4, 8]]), opac)

            # ---- main loop over gaussian tiles -------------------------------
            acc = ps.tile([128, 512], F32)
            for g in range(NGT):
                d2 = work.tile([128, 256], F32, tag="d2")
                nc.vector.tensor_scalar_add(d2[:, 0:128], idx_sb[:],
                                            negX[:, g:g + 1])
                nc.vector.tensor_scalar_add(d2[:, 128:256], idx_sb[:],
                                            negY[:, g:g + 1])
                sq = work.tile([128, 256], F32, tag="sq")
                nc.scalar.activation(sq[:], d2[:], ACTF.Square,
                                     scale=salpha[:, g:g + 1])
                w2 = work.tile([128, 256], F32, tag="w2")
                nc.scalar.activation(w2[:], sq[:], ACTF.Exp, scale=-1.0)

                c4x = work.tile([128, 512], F32, tag="c4x")
                wx = w2[:, 0:128]
                cc = col4o[:, 4 * g:4 * g + 4]
                nc.vector.tensor_mul(
                    bass.AP(c4x[:].tensor, c4x[:].offset,
                            [c4x[:].ap[0], [4, 128], [1, 4]]),
                    bass.AP(wx.tensor, wx.offset, [wx.ap[0], [1, 128], [0, 4]]),
                    bass.AP(cc.tensor, cc.offset, [cc.ap[0], [0, 128], [1, 4]]),
                )
                nc.tensor.matmul(acc[:], w2[:, 128:256], c4x[:],
                                 start=(g == 0), stop=(g == NGT - 1))

            # ---- reduce across the 4 cores of the group ----------------------
            osb = singles.tile([128, 512], F32)
            nc.scalar.copy(osb[:], acc[:])
            cc_in = dram.tile([128, 512], F32)
            nc.sync.dma_start(out=cc_in[:], in_=osb[:])
            cc_out = dram.tile([32, 512], F32)
            nc.gpsimd.collective_compute(
                "ReduceScatter", ALU.add, replica_groups=GROUPS,
                ins=[cc_in[:].opt()], outs=[cc_out[:].opt()],
            )
            res = singles.tile([32, 512], F32)
            nc.sync.dma_start(out=res[:], in_=cc_out[:])

            # ---- divide and emit this rank's 32 image rows -------------------
            dplus = singles.tile([32, 128], F32)
            nc.vector.tensor_scalar_add(dplus[:], res[:, 3::4], EPS)
            rcp = singles.tile([32, 128], F32)
            nc.vector.reciprocal(rcp[:], dplus[:])
            imgsb = singles.tile([32, 384], F32)
            for c in range(3):
                nc.vector.tensor_mul(imgsb[:, 128 * c:128 * (c + 1)],
                                     res[:, c::4], rcp[:])
                nc.sync.dma_start(out=img.ap()[c],
                                  in_=imgsb[:, 128 * c:128 * (c + 1)])

    nc.compile()
    return nc


def _quat_to_rot(q):
    q = np.asarray(q, np.float64)
    q = q / np.linalg.norm(q)
    w, x, y, z = q
    return np.array([
        [1 - 2 * (y * y + z * z), 2 * (x * y - z * w), 2 * (x * z + y * w)],
        [2 * (x * y + z * w), 1 - 2 * (x * x + z * z), 2 * (y * z - x * w)],
        [2 * (x * z - y * w), 2 * (y * z + x * w), 1 - 2 * (x * x + y * y)],
    ])


def make_in_maps(positions, colors, opacities, scales, qvec, tvec):
    in_maps = []
    for core in range(NCORES):
        b, r = core // 4, core % 4
        sl = slice(NSHARD * r, NSHARD * (r + 1))
        gdat = np.ascontiguousarray(np.stack([
            positions[sl, 0], positions[sl, 1], positions[sl, 2],
            colors[sl, 0], colors[sl, 1], colors[sl, 2],
            opacities[sl, 0], scales[sl, 0],
        ]).astype(np.float32))
        R = _quat_to_rot(qvec[b])
        cam = np.concatenate([R.reshape(9), np.asarray(tvec[b], np.float64)])
        in_maps.append({"gdat": gdat, "cam": cam.astype(np.float32)})
    return in_maps


def assemble(results):
    out = np.empty((2, 3, H, W), np.float32)
    for core in range(NCORES):
        b, r = core // 4, core % 4
        out[b, :, 32 * r:32 * (r + 1), :] = results[core]["img_part"]
    return out


def kernel(positions, colors, opacities, scales, qvec, tvec):
    global _NC_CACHE
    if _NC_CACHE is None:
        _NC_CACHE = build_nc()
    in_maps = make_in_maps(positions, colors, opacities, scales, qvec, tvec)
    r = run_bass_kernel_spmd(_NC_CACHE, in_maps, list(range(NCORES)))
    return assemble(r.results)


# revision 4
# speedup vs baseline: 1.5236x; 1.5236x over previous
"""Differentiable 3D Gaussian renderer on 8 Trainium2 NeuronCores.

Math (per batch b):
    R = quat_to_rot(qvec[b]);  p_cam = positions @ R.T + tvec[b]
    X = p_cam.x / p_cam.z * FX + CX ;  Y likewise
    w[n, p] = opacity_n * exp(-0.5 * ((px - X_n)^2 + (py - Y_n)^2) / scales_n^2)
    img[p] = (sum_n w * color_n) / (sum_n w + 1e-8)

Key restructuring for the hardware: the Gaussian is separable,
    w[n, (x, y)] = wx[n, x] * wy[n, y] * opacity_n
with wx/wy being [N, 128] / [N, 32] tables (pixel coordinates take only 128
distinct values per axis).  Per 128-gaussian tile the whole pixel-space
reduction becomes ONE [128g, 32y] x [128g, 512xc] fp32 matmul accumulated in
PSUM:
    out[y, (x, c)] += sum_n wy[n, y] * (wx[n, x] * colors4op[n, c])
with colors4op = [r*op, g*op, b*op, op] (c=3 accumulates the denominator).

Sharding: NO collectives (an 8-core collective costs ~75us in barriers on
this stack; the whole compute is ~30us).  Each core renders one batch
(core//4) for one quarter of the image rows (32 rows, base 32*(core%4)) over
ALL 4096 gaussians, and returns a [3, 32, 128] image slice.  The host stacks
the 8 slices into the [2, 3, 128, 128] output.
"""
import sys

for _p in ("/opt/trn_rl_repo", "/root/.axon_site/_ro/trn_rl_repo"):
    if _p not in sys.path:
        sys.path.append(_p)

import numpy as np

import concourse.bass as bass
import concourse.bacc as bacc
import concourse.tile as tile
from concourse import mybir
from concourse.bass_utils import run_bass_kernel_spmd

F32 = mybir.dt.float32
ALU = mybir.AluOpType
ACTF = mybir.ActivationFunctionType

H = W = 128
FX = 500.0
CX = 64.0
EPS = 1e-8
N_FULL = 4096
NCORES = 8
NSLOT = N_FULL // 128         # 32 gaussian tiles (all gaussians on every core)
GBATCH = 4                    # gaussian tiles batched per ACT Square/Exp pass
ROWS = 32                     # image rows per core

_NC_CACHE = None


def build_nc():
    nc = bacc.Bacc("TRN2", target_bir_lowering=False, debug=False,
                   num_devices=NCORES)

    # gdat[p, 8k + s] = array_k[s*128 + p] for
    # arrays (posx, posy, posz, colr, colg, colb, opac, scales); host-prepped.
    gdat = nc.dram_tensor("gdat", [128, 8 * NSLOT], F32, kind="ExternalInput")
    # cam: [R00..R22 (9), tx, ty, tz, CX, CY + 32*(core%4)]
    cam = nc.dram_tensor("cam", [14], F32, kind="ExternalInput")
    img = nc.dram_tensor("img_part", [3, ROWS, 128], F32, kind="ExternalOutput")

    idx_np = np.tile(np.arange(128, dtype=np.float32), (128, 1))
    idx_const = nc.inline_tensor(idx_np, "idxrep")

    with tile.TileContext(nc) as tc:
        with (
            tc.tile_pool(name="singles", bufs=1) as singles,
            tc.tile_pool(name="pre", bufs=1) as pre,
            tc.tile_pool(name="work", bufs=3) as work,
            tc.tile_pool(name="ps", bufs=1, space="PSUM") as ps,
        ):
            # ---- load inputs (all contiguous DMAs) ---------------------------
            gsb = singles.tile([128, 8 * NSLOT], F32)
            nc.sync.dma_start(out=gsb[:], in_=gdat.ap())
            cam_sb = singles.tile([128, 14], F32)
            c0 = cam.ap()
            nc.sync.dma_start(
                out=cam_sb[:], in_=bass.AP(c0.tensor, 0, [[0, 128], [1, 14]]))
            idx_sb = singles.tile([128, 128], F32)
            nc.sync.dma_start(out=idx_sb[:], in_=idx_const.ap())

            def gcol(k):
                return gsb[:, NSLOT * k:NSLOT * (k + 1)]

            posx, posy, posz = gcol(0), gcol(1), gcol(2)
            colr, colg, colb = gcol(3), gcol(4), gcol(5)
            opac, scal = gcol(6), gcol(7)

            def cs(i):
                return cam_sb[:, i:i + 1]

            # ---- projection preamble ([128, 32] tiles) -----------------------
            def project(name, r0, r1, r2, tt):
                t1 = pre.tile([128, NSLOT], F32, tag=f"{name}_t1")
                nc.vector.tensor_scalar(t1[:], posx, cs(r0), cs(tt),
                                        ALU.mult, ALU.add)
                t2 = pre.tile([128, NSLOT], F32, tag=f"{name}_t2")
                nc.vector.scalar_tensor_tensor(t2[:], posy, cs(r1), t1[:],
                                               ALU.mult, ALU.add)
                t3 = pre.tile([128, NSLOT], F32, tag=f"{name}_t3")
                nc.vector.scalar_tensor_tensor(t3[:], posz, cs(r2), t2[:],
                                               ALU.mult, ALU.add)
                return t3

            pcx = project("pcx", 0, 1, 2, 9)
            pcy = project("pcy", 3, 4, 5, 10)
            pcz = project("pcz", 6, 7, 8, 11)

            rz = pre.tile([128, NSLOT], F32)
            nc.vector.reciprocal(rz[:], pcz[:])

            tX = pre.tile([128, NSLOT], F32)
            nc.vector.tensor_mul(tX[:], pcx[:], rz[:])
            negX = pre.tile([128, NSLOT], F32)
            nc.vector.tensor_scalar(negX[:], tX[:], -FX, cs(12),
                                    ALU.mult, ALU.subtract)
            tY = pre.tile([128, NSLOT], F32)
            nc.vector.tensor_mul(tY[:], pcy[:], rz[:])
            negY = pre.tile([128, NSLOT], F32)
            nc.vector.tensor_scalar(negY[:], tY[:], -FX, cs(13),
                                    ALU.mult, ALU.subtract)

            rsc = pre.tile([128, NSLOT], F32)
            nc.vector.reciprocal(rsc[:], scal)
            salpha = pre.tile([128, NSLOT], F32)
            nc.vector.tensor_scalar_mul(salpha[:], rsc[:],
                                        float(np.sqrt(0.5)))

            # colors4op[p, 4s + c] = [r*op, g*op, b*op, op][c]
            col4o = pre.tile([128, 4 * NSLOT], F32)
            c4 = col4o[:]
            for off, ch in ((0, colr), (1, colg), (2, colb)):
                nc.vector.tensor_mul(
                    bass.AP(c4.tensor, c4.offset + off, [c4.ap[0], [4, NSLOT]]),
                    ch, opac)
            nc.vector.tensor_copy(
                bass.AP(c4.tensor, c4.offset + 3, [c4.ap[0], [4, NSLOT]]), opac)

            # ---- main loop over gaussian tiles -------------------------------
            # d24 batches GBATCH gtiles: per gtile 160 = [dx*salpha (128) | dy*salpha (32)]
            acc = ps.tile([ROWS, 512], F32)
            for gb in range(NSLOT // GBATCH):
                d24 = work.tile([128, 160 * GBATCH], F32, tag="d24")
                for j in range(GBATCH):
                    g = gb * GBATCH + j
                    # (idx + negX) * salpha ; (idx + negY) * salpha
                    nc.vector.tensor_scalar(
                        d24[:, 160 * j:160 * j + 128], idx_sb[:],
                        negX[:, g:g + 1], salpha[:, g:g + 1],
                        ALU.add, ALU.mult)
                    nc.vector.tensor_scalar(
                        d24[:, 160 * j + 128:160 * (j + 1)], idx_sb[:, 0:32],
                        negY[:, g:g + 1], salpha[:, g:g + 1],
                        ALU.add, ALU.mult)
                sq = work.tile([128, 160 * GBATCH], F32, tag="sq")
                nc.scalar.activation(sq[:], d24[:], ACTF.Square)
                w2 = work.tile([128, 160 * GBATCH], F32, tag="w2")
                nc.scalar.activation(w2[:], sq[:], ACTF.Exp, scale=-1.0)

                for j in range(GBATCH):
                    g = gb * GBATCH + j
                    c4x = work.tile([128, 512], F32, tag="c4x")
                    wx = w2[:, 160 * j:160 * j + 128]
                    wy = w2[:, 160 * j + 128:160 * (j + 1)]
                    cc = col4o[:, 4 * g:4 * g + 4]
                    nc.vector.tensor_mul(
                        bass.AP(c4x[:].tensor, c4x[:].offset,
                                [c4x[:].ap[0], [4, 128], [1, 4]]),
                        bass.AP(wx.tensor, wx.offset, [wx.ap[0], [1, 128], [0, 4]]),
                        bass.AP(cc.tensor, cc.offset, [cc.ap[0], [0, 128], [1, 4]]),
                    )
                    nc.tensor.matmul(acc[:], wy, c4x[:],
                                     start=(g == 0), stop=(g == NSLOT - 1))

            # ---- divide and emit this core's 32 image rows -------------------
            res = singles.tile([ROWS, 512], F32)
            nc.scalar.copy(res[:], acc[:])
            dplus = singles.tile([ROWS, 128], F32)
            nc.vector.tensor_scalar_add(dplus[:], res[:, 3::4], EPS)
            rcp = singles.tile([ROWS, 128], F32)
            nc.vector.reciprocal(rcp[:], dplus[:])
            imgsb = singles.tile([ROWS, 384], F32)
            for c in range(3):
                nc.vector.tensor_mul(imgsb[:, 128 * c:128 * (c + 1)],
                                     res[:, c::4], rcp[:])
                nc.sync.dma_start(out=img.ap()[c],
                                  in_=imgsb[:, 128 * c:128 * (c + 1)])

    nc.compile()
    return nc


def _quat_to_rot(q):
    q = np.asarray(q, np.float64)
    q = q / np.linalg.norm(q)
    w, x, y, z = q
    return np.array([
        [1 - 2 * (y * y + z * z), 2 * (x * y - z * w), 2 * (x * z + y * w)],
        [2 * (x * y + z * w), 1 - 2 * (x * x + z * z), 2 * (y * z - x * w)],
        [2 * (x * z - y * w), 2 * (y * z + x * w), 1 - 2 * (x * x + y * y)],
    ])


def make_in_maps(positions, colors, opacities, scales, qvec, tvec):
    arrs = [positions[:, 0], positions[:, 1], positions[:, 2],
            colors[:, 0], colors[:, 1], colors[:, 2],
            opacities[:, 0], scales[:, 0]]
    # gdat[p, 8k + s]... layout: columns (k, s): gdat[:, NSLOT*k + s] = arr_k[s*128:(s+1)*128]
    gdat = np.concatenate(
        [np.ascontiguousarray(a.astype(np.float32).reshape(NSLOT, 128).T)
         for a in arrs], axis=1)
    gdat = np.ascontiguousarray(gdat)
    in_maps = []
    for core in range(NCORES):
        b, q = core // 4, core % 4
        R = _quat_to_rot(qvec[b])
        cam = np.concatenate([
            R.reshape(9), np.asarray(tvec[b], np.float64),
            [CX, CX - ROWS * q],
        ])
        in_maps.append({"gdat": gdat, "cam": cam.astype(np.float32)})
    return in_maps


def assemble(results):
    out = np.empty((2, 3, H, W), np.float32)
    for core in range(NCORES):
        b, q = core // 4, core % 4
        out[b, :, ROWS * q:ROWS * (q + 1), :] = results[core]["img_part"]
    return out


def kernel(positions, colors, opacities, scales, qvec, tvec):
    global _NC_CACHE
    if _NC_CACHE is None:
        _NC_CACHE = build_nc()
    in_maps = make_in_maps(positions, colors, opacities, scales, qvec, tvec)
    r = run_bass_kernel_spmd(_NC_CACHE, in_maps, list(range(NCORES)))
    return assemble(r.results)


# revision 7
# speedup vs baseline: 2.0930x; 1.3737x over previous
"""Differentiable 3D Gaussian renderer on 8 Trainium2 NeuronCores.

Math (per batch b):
    R = quat_to_rot(qvec[b]);  p_cam = positions @ R.T + tvec[b]
    X = p_cam.x / p_cam.z * FX + CX ;  Y likewise
    w[n, p] = opacity_n * exp(-0.5 * ((px - X_n)^2 + (py - Y_n)^2) / scales_n^2)
    img[p] = (sum_n w * color_n) / (sum_n w + 1e-8)

Key restructuring for the hardware: the Gaussian is separable,
    w[n, (x, y)] = wx[n, x] * wy[n, y] * opacity_n
with wx/wy per-axis tables (pixel coordinates take only 128 distinct values
per axis).  Per 128-gaussian tile the whole pixel-space reduction becomes ONE
[128g, 128y] x [128g, 128cx] fp32 matmul accumulated in PSUM:
    out[y, (c, x)] += sum_n wy[n, y] * (wx[n, x] * colors4op[n, c])
with colors4op = [r*op, g*op, b*op, op] (c=3 accumulates the denominator).

Sharding: NO collectives (an 8-core collective costs ~75us in barriers on
this stack; the whole compute is ~25us).  Each core renders one batch
(core//4) for one 32-column x-strip (base 32*(core%4)) over ALL 4096
gaussians and ALL 128 rows, returning a [3, 128, 32] image strip.  The host
stacks the 8 strips into the [2, 3, 128, 128] output.

Engine split per 8-gaussian-tile batch (tiles laid out [x(32) | y(128)] per
gtile): two broadcast-AP tensor_tensor adds build (pix - proj) for 8 gtiles
at once (DVE), one batched Square (ACT), one broadcast alpha-multiply
(GPSIMD), one batched Exp (ACT); per gtile one [128,128] colors-expand
(DVE/GPSIMD split) and one fp32 matmul (PE).
"""
import sys

for _p in ("/opt/trn_rl_repo", "/root/.axon_site/_ro/trn_rl_repo"):
    if _p not in sys.path:
        sys.path.append(_p)

import numpy as np

import concourse.bass as bass
import concourse.bacc as bacc
import concourse.tile as tile
from concourse import mybir
from concourse.bass_utils import run_bass_kernel_spmd

F32 = mybir.dt.float32
ALU = mybir.AluOpType
ACTF = mybir.ActivationFunctionType

H = W = 128
FX = 500.0
CX = 64.0
EPS = 1e-8
N_FULL = 4096
NCORES = 8
NSLOT = N_FULL // 128         # 32 gaussian tiles (all gaussians on every core)
GBATCH = 8                    # gaussian tiles per batched Square/Exp pass
COLS = 32                     # image columns per core
GT_W = COLS + 128             # per-gtile block in batched tiles: [x(32) | y(128)]

_NC_CACHE = None


def _bcast(ap, inner):
    """[128,1] slice -> free dims [[1, n], [0, inner]] block-broadcast."""
    return bass.AP(ap.tensor, ap.offset, [ap.ap[0]] + inner)


def build_nc():
    nc = bacc.Bacc("TRN2", target_bir_lowering=False, debug=False,
                   num_devices=NCORES)

    # gdat[p, NSLOT*k + s] = array_k[s*128 + p] for
    # arrays (posx, posy, posz, colr, colg, colb, opac, scales); host-prepped.
    gdat = nc.dram_tensor("gdat", [128, 8 * NSLOT], F32, kind="ExternalInput")
    # cam: [R00..R22 (9), tx, ty, tz, CX - 32*(core%4), CY]
    cam = nc.dram_tensor("cam", [14], F32, kind="ExternalInput")
    img = nc.dram_tensor("img_part", [3, H, COLS], F32, kind="ExternalOutput")

    idx_np = np.tile(np.arange(128, dtype=np.float32), (128, 1))
    idx_const = nc.inline_tensor(idx_np, "idxrep")

    with tile.TileContext(nc) as tc:
        with (
            tc.tile_pool(name="singles", bufs=1) as singles,
            tc.tile_pool(name="pre", bufs=1) as pre,
            tc.tile_pool(name="work", bufs=2) as work,
            tc.tile_pool(name="cxp", bufs=3) as cxp,
            tc.tile_pool(name="ps", bufs=1, space="PSUM") as ps,
        ):
            # ---- load inputs (all contiguous DMAs) ---------------------------
            gsb = singles.tile([128, 8 * NSLOT], F32)
            nc.sync.dma_start(out=gsb[:], in_=gdat.ap())
            cam_sb = singles.tile([128, 14], F32)
            c0 = cam.ap()
            nc.sync.dma_start(
                out=cam_sb[:], in_=bass.AP(c0.tensor, 0, [[0, 128], [1, 14]]))
            idx_sb = singles.tile([128, 128], F32)
            nc.sync.dma_start(out=idx_sb[:], in_=idx_const.ap())

            def gcol(k):
                return gsb[:, NSLOT * k:NSLOT * (k + 1)]

            posx, posy, posz = gcol(0), gcol(1), gcol(2)
            colr, colg, colb = gcol(3), gcol(4), gcol(5)
            opac, scal = gcol(6), gcol(7)

            def cs(i):
                return cam_sb[:, i:i + 1]

            # ---- projection preamble ([128, 32] tiles) -----------------------
            def project(name, r0, r1, r2, tt):
                t1 = pre.tile([128, NSLOT], F32, tag=f"{name}_t1")
                nc.vector.tensor_scalar(t1[:], posx, cs(r0), cs(tt),
                                        ALU.mult, ALU.add)
                t2 = pre.tile([128, NSLOT], F32, tag=f"{name}_t2")
                nc.vector.scalar_tensor_tensor(t2[:], posy, cs(r1), t1[:],
                                               ALU.mult, ALU.add)
                t3 = pre.tile([128, NSLOT], F32, tag=f"{name}_t3")
                nc.vector.scalar_tensor_tensor(t3[:], posz, cs(r2), t2[:],
                                               ALU.mult, ALU.add)
                return t3

            pcx = project("pcx", 0, 1, 2, 9)
            pcy = project("pcy", 3, 4, 5, 10)
            pcz = project("pcz", 6, 7, 8, 11)

            rz = pre.tile([128, NSLOT], F32)
            nc.vector.reciprocal(rz[:], pcz[:])

            tX = pre.tile([128, NSLOT], F32)
            nc.vector.tensor_mul(tX[:], pcx[:], rz[:])
            negX = pre.tile([128, NSLOT], F32)
            nc.vector.tensor_scalar(negX[:], tX[:], -FX, cs(12),
                                    ALU.mult, ALU.subtract)
            tY = pre.tile([128, NSLOT], F32)
            nc.vector.tensor_mul(tY[:], pcy[:], rz[:])
            negY = pre.tile([128, NSLOT], F32)
            nc.vector.tensor_scalar(negY[:], tY[:], -FX, cs(13),
                                    ALU.mult, ALU.subtract)

            rsc = pre.tile([128, NSLOT], F32)
            nc.vector.reciprocal(rsc[:], scal)
            salpha = pre.tile([128, NSLOT], F32)
            nc.vector.tensor_scalar_mul(salpha[:], rsc[:],
                                        float(np.sqrt(0.5)))

            # colors4op[p, 4s + c] = [r*op, g*op, b*op, op][c]  (GPSIMD)
            col4o = pre.tile([128, 4 * NSLOT], F32)
            c4 = col4o[:]
            for off, ch in ((0, colr), (1, colg), (2, colb)):
                nc.vector.tensor_mul(
                    bass.AP(c4.tensor, c4.offset + off, [c4.ap[0], [4, NSLOT]]),
                    ch, opac)
            nc.vector.tensor_copy(
                bass.AP(c4.tensor, c4.offset + 3, [c4.ap[0], [4, NSLOT]]), opac)

            # ---- main loop over gaussian tiles -------------------------------
            acc = ps.tile([128, 128], F32)
            for gb in range(NSLOT // GBATCH):
                g0 = gb * GBATCH
                d24 = work.tile([128, GT_W * GBATCH], F32, tag="d24")
                for j in range(GBATCH):
                    g = g0 + j
                    nc.vector.tensor_scalar(
                        d24[:, GT_W * j:GT_W * j + COLS], idx_sb[:, 0:COLS],
                        negX[:, g:g + 1], salpha[:, g:g + 1],
                        ALU.add, ALU.mult)
                    nc.vector.tensor_scalar(
                        d24[:, GT_W * j + COLS:GT_W * (j + 1)], idx_sb[:],
                        negY[:, g:g + 1], salpha[:, g:g + 1],
                        ALU.add, ALU.mult)
                sq = work.tile([128, GT_W * GBATCH], F32, tag="sq")
                nc.scalar.activation(sq[:], d24[:], ACTF.Square)
                w2 = work.tile([128, GT_W * GBATCH], F32, tag="w2")
                nc.scalar.activation(w2[:], sq[:], ACTF.Exp, scale=-1.0)

                for j in range(GBATCH):
                    g = g0 + j
                    c4x = cxp.tile([128, 128], F32, tag="c4x")
                    wx = w2[:, GT_W * j:GT_W * j + COLS]
                    wy = w2[:, GT_W * j + COLS:GT_W * (j + 1)]
                    cc = col4o[:, 4 * g:4 * g + 4]
                    eng = nc.vector
                    eng.tensor_mul(
                        bass.AP(c4x[:].tensor, c4x[:].offset,
                                [c4x[:].ap[0], [COLS, 4], [1, COLS]]),
                        bass.AP(wx.tensor, wx.offset,
                                [wx.ap[0], [0, 4], [1, COLS]]),
                        bass.AP(cc.tensor, cc.offset,
                                [cc.ap[0], [1, 4], [0, COLS]]),
                    )
                    nc.tensor.matmul(acc[:], wy, c4x[:],
                                     start=(g == 0), stop=(g == NSLOT - 1))

            # ---- divide and emit this core's 32 image columns ----------------
            res = singles.tile([128, 128], F32)
            nc.scalar.copy(res[:], acc[:])
            dplus = singles.tile([128, COLS], F32)
            nc.vector.tensor_scalar_add(dplus[:], res[:, 3 * COLS:4 * COLS], EPS)
            rcp = singles.tile([128, COLS], F32)
            nc.vector.reciprocal(rcp[:], dplus[:])
            imgsb = singles.tile([128, 3 * COLS], F32)
            for c in range(3):
                nc.vector.tensor_mul(imgsb[:, COLS * c:COLS * (c + 1)],
                                     res[:, COLS * c:COLS * (c + 1)], rcp[:])
                nc.sync.dma_start(out=img.ap()[c],
                                  in_=imgsb[:, COLS * c:COLS * (c + 1)])

    nc.compile()
    return nc


def _quat_to_rot(q):
    q = np.asarray(q, np.float64)
    q = q / np.linalg.norm(q)
    w, x, y, z = q
    return np.array([
        [1 - 2 * (y * y + z * z), 2 * (x * y - z * w), 2 * (x * z + y * w)],
        [2 * (x * y + z * w), 1 - 2 * (x * x + z * z), 2 * (y * z - x * w)],
        [2 * (x * z - y * w), 2 * (y * z + x * w), 1 - 2 * (x * x + y * y)],
    ])


def make_in_maps(positions, colors, opacities, scales, qvec, tvec):
    arrs = [positions[:, 0], positions[:, 1], positions[:, 2],
            colors[:, 0], colors[:, 1], colors[:, 2],
            opacities[:, 0], scales[:, 0]]
    gdat = np.concatenate(
        [np.ascontiguousarray(a.astype(np.float32).reshape(NSLOT, 128).T)
         for a in arrs], axis=1)
    gdat = np.ascontiguousarray(gdat)
    in_maps = []
    for core in range(NCORES):
        b, q = core // 4, core % 4
        R = _quat_to_rot(qvec[b])
        cam = np.concatenate([
            R.reshape(9), np.asarray(tvec[b], np.float64),
            [CX - COLS * q, CX],
        ])
        in_maps.append({"gdat": gdat, "cam": cam.astype(np.float32)})
    return in_maps


def assemble(results):
    out = np.empty((2, 3, H, W), np.float32)
    for core in range(NCORES):
        b, q = core // 4, core % 4
        out[b, :, :, COLS * q:COLS * (q + 1)] = results[core]["img_part"]
    return out


def kernel(positions, colors, opacities, scales, qvec, tvec):
    global _NC_CACHE
    if _NC_CACHE is None:
        _NC_CACHE = build_nc()
    in_maps = make_in_maps(positions, colors, opacities, scales, qvec, tvec)
    r = run_bass_kernel_spmd(_NC_CACHE, in_maps, list(range(NCORES)))
    return assemble(r.results)
